# revision 1
# baseline (speedup 1.0000x reference)
"""Trainium2 Bass kernel for nn_L1RegressionMoEActionHead.

Data-parallel over batch: 16 batch elements -> 2 per core x 8 cores.
Only the selected expert's weights are shipped (host-sliced); scale factors
(1/sqrt(HD), sigmoid(gating)) and LayerNorm gamma/beta are folded into the
weights on the host.  All device GEMMs run in bf16 with fp32 PSUM accumulation.

Layouts on device (per core):
  Q^T/K^T produced transposed ([head_dim, tokens]) with RoPE fused
  (rot via constant pair-swap matmul R, combine on DVE).
  V produced natural ([tokens, dim]).
  Attention in transposed-score space: s^T = K_tile^T . Q^T  -> exp (no
  max-sub; scores are small by construction) -> denom via ones-matmul ->
  O^T = V^T-weighted accumulation.  No transposes needed in attention.
  Then o-proj (+bias via rank-1 matmul, +residual on DVE), LayerNorm
  (bn_stats/bn_aggr), PE-transpose of z, FFN (+bias) + ReLU.
"""

import math
import os

import numpy as np
import ml_dtypes

B = 16
T = 512
KA = 256
KT = 256
DIM = 1024
NH = 8
HD = 128
E = 8
EPS = 1e-5

NCORES = 8
BLOC = B // NCORES          # 2 batch elements per core
TOKQ = BLOC * T             # 1024 query tokens per core
TOKK = BLOC * KA            # 512 kv tokens per core (each of h_a / h_t)
NCT = DIM // 128            # 8 contraction tiles

BF16 = ml_dtypes.bfloat16

_CACHE = {}


def _rope_cos_sin(L):
    inv_freq = 1.0 / (10000.0 ** (np.arange(0, HD, 2, dtype=np.float32) / HD))
    freqs = np.arange(L, dtype=np.float32)[:, None] * inv_freq[None, :]
    emb = np.concatenate([freqs, freqs], axis=-1)   # (L, HD)
    return np.cos(emb), np.sin(emb)


def _rhat():
    # rot(q)[2i] = -q[2i+1]; rot(q)[2i+1] = q[2i]  =>  rot = R @ q
    R = np.zeros((HD, HD), dtype=np.float32)
    idx = np.arange(0, HD, 2)
    R[idx, idx + 1] = -1.0
    R[idx + 1, idx] = 1.0
    return R


def build_program():
    import concourse.bass as bass
    import concourse.mybir as mybir
    import concourse.tile as tile
    from concourse import bacc
    from contextlib import ExitStack

    f32 = mybir.dt.float32
    bf16 = mybir.dt.bfloat16
    AF = mybir.ActivationFunctionType
    ALU = mybir.AluOpType

    nc = bacc.Bacc("TRN2", target_bir_lowering=False, debug=False)

    # ---------------- DRAM parameters ----------------
    def din(name, shape, dt):
        return nc.dram_tensor(name, list(shape), dt, kind="ExternalInput")

    xT = din("xT", (DIM, TOKQ), bf16)
    xnat = din("xnat", (TOKQ, DIM), f32)
    haT = din("haT", (DIM, TOKK), bf16)
    htT = din("htT", (DIM, TOKK), bf16)

    wqaT = din("wqaT", (DIM, DIM), bf16)
    wqtT = din("wqtT", (DIM, DIM), bf16)
    wkaT = din("wkaT", (DIM, DIM), bf16)
    wktT = din("wktT", (DIM, DIM), bf16)
    wvaT = din("wvaT", (DIM, DIM), bf16)
    wvtT = din("wvtT", (DIM, DIM), bf16)
    woT = din("woT", (DIM, DIM), bf16)
    wfT = din("wfT", (DIM, DIM), bf16)

    biascols = din("biascols", (128, 4 * NH), f32)   # bqa|bqt|bka|bkt
    bva_b = din("bva_b", (128, DIM), f32)
    bvt_b = din("bvt_b", (128, DIM), f32)
    bo_row = din("bo_row", (1, DIM), bf16)
    bf_row = din("bf_row", (1, DIM), bf16)

    out_d = nc.dram_tensor("out", [TOKQ, DIM], f32, kind="ExternalOutput")

    # ---------------- inline constants ----------------
    cos_q, sin_q = _rope_cos_sin(T)         # (T, HD)
    cos_k, sin_k = _rope_cos_sin(KA)        # (KA, HD)
    cosqT = np.ascontiguousarray(cos_q.T).astype(BF16)          # (HD, T)
    sinqT = np.ascontiguousarray(sin_q.T).astype(BF16)
    coskT = np.ascontiguousarray(np.tile(cos_k.T, (1, BLOC))).astype(BF16)  # (HD, TOKK)
    sinkT = np.ascontiguousarray(np.tile(sin_k.T, (1, BLOC))).astype(BF16)

    # pack all bf16 constants into one blob: cols =
    # cosq[0:512] sinq[512:1024] cosk[1024:1536] sink[1536:2048]
    # rhatT[2048:2176] ident[2176:2304] ones[2304:2432]
    blob_bf = np.concatenate([
        cosqT, sinqT, coskT, sinkT,
        np.ascontiguousarray(_rhat().T).astype(BF16),
        np.eye(128, dtype=np.float32).astype(BF16),
        np.ones((128, 128), dtype=np.float32).astype(BF16),
    ], axis=1)
    c_blob_bf = nc.inline_tensor(np.ascontiguousarray(blob_bf), "c_blob_bf")
    # f32 blob: eps[0:1] ones[1:129]
    blob_f = np.concatenate([
        np.full((128, 1), EPS, dtype=np.float32),
        np.ones((128, 128), dtype=np.float32),
    ], axis=1)
    c_blob_f = nc.inline_tensor(np.ascontiguousarray(blob_f), "c_blob_f")

    with tile.TileContext(nc) as tc, ExitStack() as ctx:
        persist = ctx.enter_context(tc.tile_pool(name="persist", bufs=1))
        consts = ctx.enter_context(tc.tile_pool(name="consts", bufs=1))

        def cload(dram, shape, dt, tag):
            t = consts.tile(list(shape), dt, name=tag, tag=tag)
            nc.sync.dma_start(t[:], dram.ap())
            return t

        sb_cb = cload(c_blob_bf, (128, blob_bf.shape[1]), bf16, "cb")
        sb_cf = cload(c_blob_f, (128, blob_f.shape[1]), f32, "cf")
        sb_bias = cload(biascols, (128, 4 * NH), f32, "biasc")
        sb_cosq = sb_cb[:, 0:512]
        sb_sinq = sb_cb[:, 512:1024]
        sb_cosk = sb_cb[:, 1024:1536]
        sb_sink = sb_cb[:, 1536:2048]
        sb_rhatT = sb_cb[:, 2048:2176]
        sb_ident = sb_cb[:, 2176:2304]
        sb_ones_col = sb_cb[:, 2304:2305]
        sb_ones_row = sb_cb[0:1, 2304:2432]
        sb_ones_row_f = sb_cf[0:1, 1:129]
        sb_eps = sb_cf[:, 0:1]
        sb_bqa = sb_bias[:, 0:NH]
        sb_bqt = sb_bias[:, NH:2 * NH]
        sb_bka = sb_bias[:, 2 * NH:3 * NH]
        sb_bkt = sb_bias[:, 3 * NH:4 * NH]
        sb_bo = cload(bo_row, (1, DIM), bf16, "bo")
        sb_bf = cload(bf_row, (1, DIM), bf16, "bf")

        # persistent activation tiles
        qa_sb = [persist.tile([HD, TOKQ], bf16, name=f"qa{h}", tag=f"qa{h}") for h in range(NH)]
        qt_sb = [persist.tile([HD, TOKQ], bf16, name=f"qt{h}", tag=f"qt{h}") for h in range(NH)]
        ka_sb = [persist.tile([HD, TOKK], bf16, name=f"ka{h}", tag=f"ka{h}") for h in range(NH)]
        kt_sb = [persist.tile([HD, TOKK], bf16, name=f"kt{h}", tag=f"kt{h}") for h in range(NH)]
        va_sb = [persist.tile([128, DIM], bf16, name=f"va{i}", tag=f"va{i}") for i in range(TOKK // 128)]
        vt_sb = [persist.tile([128, DIM], bf16, name=f"vt{i}", tag=f"vt{i}") for i in range(TOKK // 128)]
        o_sb = {}
        for b in range(BLOC):
            for h in range(NH):
                o_sb[(b, h)] = persist.tile([HD, T], bf16, name=f"o{b}_{h}", tag=f"o{b}_{h}")

        # ================= Phase A: projections =================
        with tc.tile_pool(name="acts", bufs=1) as actp, \
             tc.tile_pool(name="wpool", bufs=2) as wpool, \
             tc.tile_pool(name="ptmp", bufs=3) as ptmp, \
             tc.tile_pool(name="ppsum", bufs=3, space="PSUM") as ppsum, \
             tc.tile_pool(name="rpsum", bufs=2, space="PSUM") as rpsum:

            sb_xT = actp.tile([128, NCT, TOKQ], bf16, tag="xT")
            nc.sync.dma_start(sb_xT[:], xT.ap().rearrange("(a p) t -> p a t", p=128))
            sb_haT = actp.tile([128, NCT, TOKK], bf16, tag="haT")
            nc.sync.dma_start(sb_haT[:], haT.ap().rearrange("(a p) t -> p a t", p=128))
            sb_htT = actp.tile([128, NCT, TOKK], bf16, tag="htT")
            nc.sync.dma_start(sb_htT[:], htT.ap().rearrange("(a p) t -> p a t", p=128))
            sb_bva = actp.tile([128, DIM], f32, name="bva", tag="bva")
            nc.sync.dma_start(sb_bva[:], bva_b.ap())
            sb_bvt = actp.tile([128, DIM], f32, name="bvt", tag="bvt")
            nc.sync.dma_start(sb_bvt[:], bvt_b.ap())

            def load_w(wdram):
                t = wpool.tile([128, NCT, DIM], bf16, name="w", tag="w")
                nc.sync.dma_start(
                    t[:], wdram.ap().rearrange("(a p) j -> p a j", p=128))
                return [t[:, ct, :] for ct in range(NCT)]

            def qk_stage(wdram, bias_sb, src_sb, tok_len, out_tiles, costab, sintab):
                w = load_w(wdram)
                nchunks = tok_len // 512
                for j in range(NH):
                    for ch in range(nchunks):
                        sl = slice(ch * 512, (ch + 1) * 512)
                        ps = ppsum.tile([128, 512], f32, tag="proj")
                        for ct in range(NCT):
                            nc.tensor.matmul(
                                ps[:], w[ct][:, j * 128:(j + 1) * 128],
                                src_sb[:, ct, sl],
                                start=(ct == 0), stop=(ct == NCT - 1))
                        q1 = ptmp.tile([128, 512], bf16, tag="q1")
                        nc.scalar.activation(q1[:], ps[:], AF.Identity,
                                             bias=bias_sb[:, j:j + 1])
                        rot = rpsum.tile([128, 512], f32, tag="rot")
                        nc.tensor.matmul(rot[:], sb_rhatT[:], q1[:],
                                         start=True, stop=True)
                        if tok_len == T * BLOC and nchunks == BLOC:
                            ctab = costab[:, 0:512]
                            stab = sintab[:, 0:512]
                        else:
                            ctab = costab[:, sl]
                            stab = sintab[:, sl]
                        t1 = ptmp.tile([128, 512], bf16, tag="t1")
                        nc.vector.tensor_tensor(t1[:], q1[:], ctab, op=ALU.mult)
                        t2 = ptmp.tile([128, 512], bf16, tag="t2")
                        nc.vector.tensor_tensor(t2[:], rot[:], stab, op=ALU.mult)
                        nc.vector.tensor_tensor(out_tiles[j][:, sl], t1[:], t2[:],
                                                op=ALU.add)

            def v_stage(wdram, src_sb, out_tiles, bias_bcast):
                w = load_w(wdram)
                for kt_i in range(TOKK // 128):
                    for jc in range(2):
                        sl = slice(jc * 512, (jc + 1) * 512)
                        ps = ppsum.tile([128, 512], f32, tag="proj")
                        for ct in range(NCT):
                            nc.tensor.matmul(
                                ps[:], src_sb[:, ct, kt_i * 128:(kt_i + 1) * 128],
                                w[ct][:, sl],
                                start=(ct == 0), stop=(ct == NCT - 1))
                        nc.vector.tensor_tensor(out_tiles[kt_i][:, sl], ps[:],
                                                bias_bcast[:, sl], op=ALU.add)

            # Q chunks are per-batch (512 tokens), rope tables repeat per batch.
            qk_stage(wqaT, sb_bqa, sb_xT, TOKQ, qa_sb, sb_cosq, sb_sinq)
            qk_stage(wqtT, sb_bqt, sb_xT, TOKQ, qt_sb, sb_cosq, sb_sinq)
            qk_stage(wkaT, sb_bka, sb_haT, TOKK, ka_sb, sb_cosk, sb_sink)
            v_stage(wvaT, sb_haT, va_sb, sb_bva)
            qk_stage(wktT, sb_bkt, sb_htT, TOKK, kt_sb, sb_cosk, sb_sink)
            v_stage(wvtT, sb_htT, vt_sb, sb_bvt)

        # ================= Phase B: attention =================
        with tc.tile_pool(name="atmp", bufs=6) as atmp, \
             tc.tile_pool(name="artmp", bufs=3) as artmp, \
             tc.tile_pool(name="aps", bufs=2, space="PSUM") as aps:
            for b in range(BLOC):
                for h in range(NH):
                    den = aps.tile([1, 512], f32, tag="den")
                    ov = aps.tile([128, 512], f32, tag="ov")
                    qsl = slice(b * T, (b + 1) * T)
                    for ci in range(4):
                        if ci < 2:
                            ksb, qsb, vtiles = ka_sb[h], qa_sb[h], va_sb
                            koff = b * KA + ci * 128
                            vti = b * (KA // 128) + ci
                        else:
                            ksb, qsb, vtiles = kt_sb[h], qt_sb[h], vt_sb
                            koff = b * KT + (ci - 2) * 128
                            vti = b * (KT // 128) + (ci - 2)
                        s = aps.tile([128, 512], f32, tag="s")
                        nc.tensor.matmul(s[:], ksb[:, koff:koff + 128],
                                         qsb[:, qsl], start=True, stop=True)
                        p = atmp.tile([128, 512], bf16, tag="p")
                        nc.scalar.activation(p[:], s[:], AF.Exp)
                        nc.tensor.matmul(den[:], sb_ones_col[:], p[:],
                                         start=(ci == 0), stop=(ci == 3),
                                         skip_group_check=True)
                        nc.tensor.matmul(ov[:], vtiles[vti][:, h * 128:(h + 1) * 128],
                                         p[:], start=(ci == 0), stop=(ci == 3),
                                         skip_group_check=True)
                    recip = artmp.tile([1, 512], f32, tag="recip")
                    nc.vector.reciprocal_approx_fast(recip[:], den[:])
                    recip_bf = artmp.tile([1, 512], bf16, tag="recip_bf")
                    nc.vector.tensor_copy(recip_bf[:], recip[:])
                    rbps = aps.tile([128, 512], f32, tag="rbps")
                    nc.tensor.matmul(rbps[:], sb_ones_row[:], recip_bf[:],
                                     start=True, stop=True)
                    rb = artmp.tile([128, 512], f32, tag="rb")
                    nc.scalar.activation(rb[:], rbps[:], AF.Copy)
                    nc.vector.tensor_tensor(o_sb[(b, h)][:], ov[:], rb[:],
                                            op=ALU.mult)

        # ================= Phase C: o-proj + LN + FFN =================
        with tc.tile_pool(name="w2", bufs=1) as w2, \
             tc.tile_pool(name="ctmp", bufs=2) as ctmp, \
             tc.tile_pool(name="cres", bufs=3) as cres, \
             tc.tile_pool(name="cps", bufs=2, space="PSUM") as cps:

            wot = w2.tile([128, NCT, DIM], bf16, name="wot", tag="wo")
            nc.sync.dma_start(wot[:], woT.ap().rearrange("(a p) j -> p a j", p=128))
            wo = [wot[:, ct, :] for ct in range(NCT)]
            wft = w2.tile([128, NCT, DIM], bf16, name="wft", tag="wf")
            nc.sync.dma_start(wft[:], wfT.ap().rearrange("(a p) j -> p a j", p=128))
            wf = [wft[:, ct, :] for ct in range(NCT)]

            for b in range(BLOC):
                for t4 in range(T // 128):
                    tt = b * (T // 128) + t4
                    row0 = tt * 128
                    xn = ctmp.tile([128, DIM], f32, tag="xn")
                    nc.sync.dma_start(xn[:], xnat.ap()[row0:row0 + 128, :])
                    x2t = ctmp.tile([128, DIM], f32, tag="x2")
                    # o-proj into x2 (+bias via rank-1, +residual on DVE)
                    for jc in range(2):
                        sl = slice(jc * 512, (jc + 1) * 512)
                        ps = cps.tile([128, 512], f32, tag="op")
                        for h in range(NH):
                            nc.tensor.matmul(
                                ps[:], o_sb[(b, h)][:, t4 * 128:(t4 + 1) * 128],
                                wo[h][:, sl], start=(h == 0), stop=False)
                        nc.tensor.matmul(ps[:], sb_ones_row[:], sb_bo[:, sl],
                                         start=False, stop=True)
                        nc.vector.tensor_tensor(x2t[:, sl], ps[:],
                                                xn[:, sl], op=ALU.add)
                    # LayerNorm stats
                    stats = ctmp.tile([128, 2, 6], f32, tag="stats")
                    nc.vector.bn_stats(stats[:, 0, :], x2t[:, 0:512])
                    nc.vector.bn_stats(stats[:, 1, :], x2t[:, 512:1024])
                    mv = ctmp.tile([128, 2], f32, tag="mv")
                    nc.vector.bn_aggr(mv[:], stats[:])
                    rstd = ctmp.tile([128, 1], f32, tag="rstd")
                    nc.scalar.activation(rstd[:], mv[:, 1:2], AF.Sqrt,
                                         bias=sb_eps[:])
                    rstd2 = ctmp.tile([128, 1], f32, tag="rstd2")
                    nc.vector.reciprocal(rstd2[:], rstd[:])
                    z = ctmp.tile([128, DIM], bf16, tag="z")
                    nc.vector.tensor_scalar(z[:], x2t[:],
                                            scalar1=mv[:, 0:1], scalar2=rstd2[:],
                                            op0=ALU.subtract, op1=ALU.mult)
                    # transpose z -> zT (2 halves of 4 blocks each)
                    zT = []
                    for half in range(2):
                        tp = cps.tile([128, 512], bf16, tag="tp")
                        for q in range(4):
                            cb = half * 4 + q
                            nc.tensor.transpose(
                                tp[:, q * 128:(q + 1) * 128],
                                z[:, cb * 128:(cb + 1) * 128], sb_ident[:])
                        zt = ctmp.tile([128, 512], bf16, tag=f"zT{half}")
                        nc.vector.tensor_copy(zt[:], tp[:])
                        zT.append(zt)
                    # FFN + ReLU + store
                    for jc in range(2):
                        sl = slice(jc * 512, (jc + 1) * 512)
                        fp = cps.tile([128, 512], f32, tag="fp")
                        for ct in range(NCT):
                            nc.tensor.matmul(
                                fp[:], zT[ct // 4][:, (ct % 4) * 128:(ct % 4 + 1) * 128],
                                wf[ct][:, sl], start=(ct == 0), stop=False)
                        nc.tensor.matmul(fp[:], sb_ones_row[:], sb_bf[:, sl],
                                         start=False, stop=True)
                        res = cres.tile([128, 512], f32, tag="res")
                        nc.scalar.activation(res[:], fp[:], AF.Relu)
                        nc.sync.dma_start(out_d.ap()[row0:row0 + 128, sl], res[:])

    nc.compile()
    return nc


def _prep_host(inputs):
    """Host-side preprocessing: expert select, folding, transposes, sharding."""
    x = np.asarray(inputs["x"], dtype=np.float32)
    h_a = np.asarray(inputs["h_a"], dtype=np.float32)
    h_t = np.asarray(inputs["h_t"], dtype=np.float32)
    e = int(np.asarray(inputs["expert_idx"]))
    g = float(1.0 / (1.0 + math.exp(-float(np.asarray(inputs["gating_factor"])[e]))))
    sc = 1.0 / math.sqrt(HD)

    def wT(w, scale=1.0):
        return np.ascontiguousarray(
            (np.asarray(w, dtype=np.float32)[e] * scale).T).astype(BF16)

    def brow(bv, scale=1.0, dtype=BF16):
        return (np.asarray(bv, dtype=np.float32)[e] * scale).reshape(1, DIM).astype(dtype)

    def bcol(bv, scale=1.0):
        # [DIM] -> [128, NH]: column h = b[h*128:(h+1)*128]
        return np.ascontiguousarray(
            (np.asarray(bv, dtype=np.float32)[e] * scale).reshape(NH, 128).T
        ).astype(np.float32)

    gamma = np.asarray(inputs["gamma"], dtype=np.float32)[e]
    beta = np.asarray(inputs["beta"], dtype=np.float32)[e]
    w_ffn = np.asarray(inputs["W_ffn"], dtype=np.float32)[e]
    b_ffn = np.asarray(inputs["b_ffn"], dtype=np.float32)[e]
    w_f_eff = w_ffn * gamma[None, :]
    b_f_eff = b_ffn + w_ffn @ beta

    shared = {
        "wqaT": wT(inputs["W_qa"], sc),
        "wqtT": wT(inputs["W_qt"], sc * g),
        "wkaT": wT(inputs["W_ka"]),
        "wktT": wT(inputs["W_kt"]),
        "wvaT": wT(inputs["W_va"]),
        "wvtT": wT(inputs["W_vt"]),
        "woT": wT(inputs["W_o"]),
        "wfT": np.ascontiguousarray(w_f_eff.T).astype(BF16),
        "biascols": np.ascontiguousarray(np.concatenate([
            bcol(inputs["b_qa"], sc),
            bcol(inputs["b_qt"], sc * g),
            bcol(inputs["b_ka"]),
            bcol(inputs["b_kt"]),
        ], axis=1)),
        "bva_b": np.ascontiguousarray(np.tile(
            np.asarray(inputs["b_va"], dtype=np.float32)[e][None, :], (128, 1))),
        "bvt_b": np.ascontiguousarray(np.tile(
            np.asarray(inputs["b_vt"], dtype=np.float32)[e][None, :], (128, 1))),
        "bo_row": brow(inputs["b_o"]),
        "bf_row": b_f_eff.reshape(1, DIM).astype(BF16),
    }

    in_maps = []
    for c in range(NCORES):
        xc = x[c * BLOC:(c + 1) * BLOC].reshape(TOKQ, DIM)
        hac = h_a[c * BLOC:(c + 1) * BLOC].reshape(TOKK, DIM)
        htc = h_t[c * BLOC:(c + 1) * BLOC].reshape(TOKK, DIM)
        m = dict(shared)
        m["xT"] = np.ascontiguousarray(xc.T).astype(BF16)
        m["xnat"] = np.ascontiguousarray(xc)
        m["haT"] = np.ascontiguousarray(hac.T).astype(BF16)
        m["htT"] = np.ascontiguousarray(htc.T).astype(BF16)
        in_maps.append(m)
    return in_maps


def run(inputs, trace=False):
    from concourse.bass_utils import run_bass_kernel_spmd

    if "nc" not in _CACHE:
        _CACHE["nc"] = build_program()
    nc = _CACHE["nc"]
    in_maps = _prep_host(inputs)
    res = run_bass_kernel_spmd(nc, in_maps, list(range(NCORES)), trace=trace)
    outs = [res.results[c]["out"].reshape(BLOC, T, DIM) for c in range(NCORES)]
    return np.concatenate(outs, axis=0), res


def kernel(**inputs) -> np.ndarray:
    out, _ = run(inputs, trace=False)
    return out



# revision 12
# speedup vs baseline: 1.1151x; 1.1151x over previous
"""Trainium2 Bass kernel for nn_L1RegressionMoEActionHead.

Data-parallel over batch: 16 batch elements -> 2 per core x 8 cores.
Only the selected expert's weights are shipped (host-sliced); scale factors
(1/sqrt(HD), sigmoid(gating)) and LayerNorm gamma/beta are folded into the
weights on the host.  All device GEMMs run in bf16 with fp32 PSUM accumulation.

Layouts on device (per core):
  Q^T/K^T produced transposed ([head_dim, tokens]) with RoPE fused
  (rot via constant pair-swap matmul R, combine on DVE).
  V produced natural ([tokens, dim]).
  Attention in transposed-score space: s^T = K_tile^T . Q^T  -> exp (no
  max-sub; scores are small by construction) -> denom via ones-matmul ->
  O^T = V^T-weighted accumulation.  No transposes needed in attention.
  Then o-proj (+bias via rank-1 matmul, +residual on DVE), LayerNorm
  (bn_stats/bn_aggr), PE-transpose of z, FFN (+bias) + ReLU.
"""

import math
import os

import numpy as np
import ml_dtypes

B = 16
T = 512
KA = 256
KT = 256
DIM = 1024
NH = 8
HD = 128
E = 8
EPS = 1e-5

NCORES = 8
BLOC = B // NCORES          # 2 batch elements per core
TOKQ = BLOC * T             # 1024 query tokens per core
TOKK = BLOC * KA            # 512 kv tokens per core (each of h_a / h_t)
NCT = DIM // 128            # 8 contraction tiles

BF16 = ml_dtypes.bfloat16

_CACHE = {}


def _rope_cos_sin(L):
    inv_freq = 1.0 / (10000.0 ** (np.arange(0, HD, 2, dtype=np.float32) / HD))
    freqs = np.arange(L, dtype=np.float32)[:, None] * inv_freq[None, :]
    emb = np.concatenate([freqs, freqs], axis=-1)   # (L, HD)
    return np.cos(emb), np.sin(emb)


def _rhat():
    # rot(q)[2i] = -q[2i+1]; rot(q)[2i+1] = q[2i]  =>  rot = R @ q
    R = np.zeros((HD, HD), dtype=np.float32)
    idx = np.arange(0, HD, 2)
    R[idx, idx + 1] = -1.0
    R[idx + 1, idx] = 1.0
    return R


def build_program():
    import concourse.bass as bass
    import concourse.mybir as mybir
    import concourse.tile as tile
    from concourse import bacc
    from contextlib import ExitStack

    f32 = mybir.dt.float32
    bf16 = mybir.dt.bfloat16
    AF = mybir.ActivationFunctionType
    ALU = mybir.AluOpType

    nc = bacc.Bacc("TRN2", target_bir_lowering=False, debug=False)

    # ---------------- DRAM parameters ----------------
    def din(name, shape, dt):
        return nc.dram_tensor(name, list(shape), dt, kind="ExternalInput")

    xT = din("xT", (DIM, TOKQ), bf16)
    xnat = din("xnat", (TOKQ, DIM), f32)
    haT = din("haT", (DIM, TOKK), bf16)
    htT = din("htT", (DIM, TOKK), bf16)

    wqaT = din("wqaT", (DIM, DIM), bf16)
    wqtT = din("wqtT", (DIM, DIM), bf16)
    wkaT = din("wkaT", (DIM, DIM), bf16)
    wktT = din("wktT", (DIM, DIM), bf16)
    wvaT = din("wvaT", (DIM, DIM), bf16)
    wvtT = din("wvtT", (DIM, DIM), bf16)
    woT = din("woT", (DIM, DIM), bf16)
    wfT = din("wfT", (DIM, DIM), bf16)

    biascols = din("biascols", (128, 4 * NH), f32)   # bqa|bqt|bka|bkt
    bva_b = din("bva_b", (128, DIM), f32)
    bvt_b = din("bvt_b", (128, DIM), f32)
    bf_row = din("bf_row", (1, DIM), bf16)

    out_d = nc.dram_tensor("out", [TOKQ, DIM], f32, kind="ExternalOutput")

    # ---------------- inline constants ----------------
    cos_q, sin_q = _rope_cos_sin(T)         # (T, HD)
    cos_k, sin_k = _rope_cos_sin(KA)        # (KA, HD)
    cosqT = np.ascontiguousarray(cos_q.T).astype(BF16)          # (HD, T)
    sinqT = np.ascontiguousarray(sin_q.T).astype(BF16)
    coskT = np.ascontiguousarray(np.tile(cos_k.T, (1, BLOC))).astype(BF16)  # (HD, TOKK)
    sinkT = np.ascontiguousarray(np.tile(sin_k.T, (1, BLOC))).astype(BF16)

    # pack all bf16 constants into one blob: cols =
    # cosq[0:512] sinq[512:1024] cosk[1024:1536] sink[1536:2048]
    # rhatT[2048:2176] ident[2176:2304] ones[2304:2432]
    blob_bf = np.concatenate([
        cosqT, sinqT, coskT, sinkT,
        np.ascontiguousarray(_rhat().T).astype(BF16),
        np.eye(128, dtype=np.float32).astype(BF16),
        np.ones((128, 128), dtype=np.float32).astype(BF16),
    ], axis=1)
    c_blob_bf = nc.inline_tensor(np.ascontiguousarray(blob_bf), "c_blob_bf")
    # f32 blob: eps[0:1] ones[1:129]
    blob_f = np.concatenate([
        np.full((128, 1), EPS, dtype=np.float32),
        np.ones((128, 128), dtype=np.float32),
    ], axis=1)
    c_blob_f = nc.inline_tensor(np.ascontiguousarray(blob_f), "c_blob_f")

    with tile.TileContext(nc) as tc, ExitStack() as ctx:
        persist = ctx.enter_context(tc.tile_pool(name="persist", bufs=1))
        consts = ctx.enter_context(tc.tile_pool(name="consts", bufs=1))

        def cload(dram, shape, dt, tag):
            t = consts.tile(list(shape), dt, name=tag, tag=tag)
            nc.sync.dma_start(t[:], dram.ap())
            return t

        sb_cb = cload(c_blob_bf, (128, blob_bf.shape[1]), bf16, "cb")
        sb_cf = cload(c_blob_f, (128, blob_f.shape[1]), f32, "cf")
        sb_bias = cload(biascols, (128, 4 * NH), f32, "biasc")
        sb_cosq = sb_cb[:, 0:512]
        sb_sinq = sb_cb[:, 512:1024]
        sb_cosk = sb_cb[:, 1024:1536]
        sb_sink = sb_cb[:, 1536:2048]
        sb_rhatT = sb_cb[:, 2048:2176]
        sb_ident = sb_cb[:, 2176:2304]
        sb_ones_col = sb_cb[:, 2304:2305]
        sb_ones_row = sb_cb[0:1, 2304:2432]
        sb_ones_row_f = sb_cf[0:1, 1:129]
        sb_eps = sb_cf[:, 0:1]
        sb_bqa = sb_bias[:, 0:NH]
        sb_bqt = sb_bias[:, NH:2 * NH]
        sb_bka = sb_bias[:, 2 * NH:3 * NH]
        sb_bkt = sb_bias[:, 3 * NH:4 * NH]
        sb_bf = cload(bf_row, (1, DIM), bf16, "bf")

        # persistent activation tiles
        qa_sb = [persist.tile([HD, TOKQ], bf16, name=f"qa{h}", tag=f"qa{h}") for h in range(NH)]
        qt_sb = [persist.tile([HD, TOKQ], bf16, name=f"qt{h}", tag=f"qt{h}") for h in range(NH)]
        ka_sb = [persist.tile([HD, TOKK], bf16, name=f"ka{h}", tag=f"ka{h}") for h in range(NH)]
        kt_sb = [persist.tile([HD, TOKK], bf16, name=f"kt{h}", tag=f"kt{h}") for h in range(NH)]
        va_sb = [persist.tile([128, DIM], bf16, name=f"va{i}", tag=f"va{i}") for i in range(TOKK // 128)]
        vt_sb = [persist.tile([128, DIM], bf16, name=f"vt{i}", tag=f"vt{i}") for i in range(TOKK // 128)]
        o_sb = {}
        for b in range(BLOC):
            for h in range(NH):
                o_sb[(b, h)] = persist.tile([HD, T], bf16, name=f"o{b}_{h}", tag=f"o{b}_{h}")

        # w2 pool created early so wo/wf prefetch overlaps Phases A/B
        w2 = ctx.enter_context(tc.tile_pool(name="w2", bufs=1))

        # ================= Phase A: projections =================
        with tc.tile_pool(name="acts", bufs=1) as actp, \
             tc.tile_pool(name="wpool", bufs=2) as wpool, \
             tc.tile_pool(name="ptmp", bufs=3) as ptmp, \
             tc.tile_pool(name="warm", bufs=1, space="PSUM") as warmp, \
             tc.tile_pool(name="ppsum", bufs=3, space="PSUM") as ppsum, \
             tc.tile_pool(name="rpsum", bufs=2, space="PSUM") as rpsum:

            # critical-path DMAs first: xT, then wqa (split in halves so
            # heads 0-3 can start before the whole weight lands)
            sb_xT = actp.tile([128, NCT, TOKQ], bf16, tag="xT")
            nc.sync.dma_start(sb_xT[:], xT.ap().rearrange("(a p) t -> p a t", p=128))

            # warm up the PE clock (HAM) while input DMAs stream
            wsink = warmp.tile([128, 512], f32, tag="wsink")
            for _ in range(36):
                nc.tensor.matmul(wsink[:], sb_ident[:], sb_cb[:, 0:512],
                                 start=True, stop=True)

            def load_w(wdram, split=False):
                t = wpool.tile([128, NCT, DIM], bf16, name="w", tag="w")
                src = wdram.ap().rearrange("(a p) j -> p a j", p=128)
                if split:
                    nc.sync.dma_start(t[:, :, 0:512], src[:, :, 0:512])
                    nc.sync.dma_start(t[:, :, 512:1024], src[:, :, 512:1024])
                else:
                    nc.sync.dma_start(t[:], src)
                return [t[:, ct, :] for ct in range(NCT)]

            def qk_stage(wdram, bias_sb, src_sb, tok_len, out_tiles, costab,
                         sintab, split=False):
                w = load_w(wdram, split=split)
                nchunks = tok_len // 512
                for j in range(NH):
                    for ch in range(nchunks):
                        sl = slice(ch * 512, (ch + 1) * 512)
                        ps = ppsum.tile([128, 512], f32, tag="proj")
                        for ct in range(NCT):
                            nc.tensor.matmul(
                                ps[:], w[ct][:, j * 128:(j + 1) * 128],
                                src_sb[:, ct, sl],
                                start=(ct == 0), stop=(ct == NCT - 1))
                        q1 = ptmp.tile([128, 512], bf16, tag="q1")
                        nc.scalar.activation(q1[:], ps[:], AF.Identity,
                                             bias=bias_sb[:, j:j + 1])
                        rot = rpsum.tile([128, 512], f32, tag="rot")
                        nc.tensor.matmul(rot[:], sb_rhatT[:], q1[:],
                                         start=True, stop=True)
                        if tok_len == T * BLOC and nchunks == BLOC:
                            ctab = costab[:, 0:512]
                            stab = sintab[:, 0:512]
                        else:
                            ctab = costab[:, sl]
                            stab = sintab[:, sl]
                        t1 = ptmp.tile([128, 512], bf16, tag="t1")
                        nc.vector.tensor_tensor(t1[:], q1[:], ctab, op=ALU.mult)
                        t2 = ptmp.tile([128, 512], bf16, tag="t2")
                        nc.vector.tensor_tensor(t2[:], rot[:], stab, op=ALU.mult)
                        nc.vector.tensor_tensor(out_tiles[j][:, sl], t1[:], t2[:],
                                                op=ALU.add)

            def v_stage(wdram, src_sb, out_tiles, bias_bcast):
                w = load_w(wdram)
                for kt_i in range(TOKK // 128):
                    for jc in range(2):
                        sl = slice(jc * 512, (jc + 1) * 512)
                        ps = ppsum.tile([128, 512], f32, tag="proj")
                        for ct in range(NCT):
                            nc.tensor.matmul(
                                ps[:], src_sb[:, ct, kt_i * 128:(kt_i + 1) * 128],
                                w[ct][:, sl],
                                start=(ct == 0), stop=(ct == NCT - 1))
                        nc.vector.tensor_tensor(out_tiles[kt_i][:, sl], ps[:],
                                                bias_bcast[:, sl], op=ALU.add)

            # Q chunks are per-batch (512 tokens), rope tables repeat per batch.
            qk_stage(wqaT, sb_bqa, sb_xT, TOKQ, qa_sb, sb_cosq, sb_sinq,
                     split=True)
            # remaining input DMAs issue here, behind the critical-path ones
            sb_haT = actp.tile([128, NCT, TOKK], bf16, tag="haT")
            nc.sync.dma_start(sb_haT[:], haT.ap().rearrange("(a p) t -> p a t", p=128))
            sb_htT = actp.tile([128, NCT, TOKK], bf16, tag="htT")
            nc.sync.dma_start(sb_htT[:], htT.ap().rearrange("(a p) t -> p a t", p=128))
            sb_bva = actp.tile([128, DIM], f32, name="bva", tag="bva")
            nc.sync.dma_start(sb_bva[:], bva_b.ap())
            sb_bvt = actp.tile([128, DIM], f32, name="bvt", tag="bvt")
            nc.sync.dma_start(sb_bvt[:], bvt_b.ap())

            qk_stage(wqtT, sb_bqt, sb_xT, TOKQ, qt_sb, sb_cosq, sb_sinq)
            qk_stage(wkaT, sb_bka, sb_haT, TOKK, ka_sb, sb_cosk, sb_sink)
            v_stage(wvaT, sb_haT, va_sb, sb_bva)
            qk_stage(wktT, sb_bkt, sb_htT, TOKK, kt_sb, sb_cosk, sb_sink)
            v_stage(wvtT, sb_htT, vt_sb, sb_bvt)

            # prefetch Phase C weights; lands well before attention finishes
            wot = w2.tile([128, NCT, DIM], bf16, name="wot", tag="wo")
            nc.sync.dma_start(wot[:], woT.ap().rearrange("(a p) j -> p a j", p=128))
            wft = w2.tile([128, NCT, DIM], bf16, name="wft", tag="wf")
            nc.sync.dma_start(wft[:], wfT.ap().rearrange("(a p) j -> p a j", p=128))

        # residual (x + b_o), loaded once; DMA overlaps Phase B
        xn_pool = ctx.enter_context(tc.tile_pool(name="xnp", bufs=1))
        xn_all = xn_pool.tile([128, TOKQ // 128, DIM], f32, tag="xn")
        nc.sync.dma_start(xn_all[:],
                          xnat.ap().rearrange("(tt p) d -> p tt d", p=128))

        # ================= Phase B: attention =================
        with tc.tile_pool(name="atmp", bufs=6) as atmp, \
             tc.tile_pool(name="artmp", bufs=3) as artmp, \
             tc.tile_pool(name="aps", bufs=2, space="PSUM") as aps:
            for b in range(BLOC):
                for h in range(NH):
                    den = aps.tile([1, 512], f32, tag="den")
                    ov = aps.tile([128, 512], f32, tag="ov")
                    qsl = slice(b * T, (b + 1) * T)
                    for ci in range(4):
                        if ci < 2:
                            ksb, qsb, vtiles = ka_sb[h], qa_sb[h], va_sb
                            koff = b * KA + ci * 128
                            vti = b * (KA // 128) + ci
                        else:
                            ksb, qsb, vtiles = kt_sb[h], qt_sb[h], vt_sb
                            koff = b * KT + (ci - 2) * 128
                            vti = b * (KT // 128) + (ci - 2)
                        s = aps.tile([128, 512], f32, tag="s")
                        nc.tensor.matmul(s[:], ksb[:, koff:koff + 128],
                                         qsb[:, qsl], start=True, stop=True)
                        p = atmp.tile([128, 512], bf16, tag="p")
                        nc.scalar.activation(p[:], s[:], AF.Exp)
                        nc.tensor.matmul(den[:], sb_ones_col[:], p[:],
                                         start=(ci == 0), stop=(ci == 3),
                                         skip_group_check=True)
                        nc.tensor.matmul(ov[:], vtiles[vti][:, h * 128:(h + 1) * 128],
                                         p[:], start=(ci == 0), stop=(ci == 3),
                                         skip_group_check=True)
                    recip = artmp.tile([1, 512], f32, tag="recip")
                    nc.vector.reciprocal_approx_fast(recip[:], den[:])
                    recip_bf = artmp.tile([1, 512], bf16, tag="recip_bf")
                    nc.vector.tensor_copy(recip_bf[:], recip[:])
                    rbps = aps.tile([128, 512], f32, tag="rbps")
                    nc.tensor.matmul(rbps[:], sb_ones_row[:], recip_bf[:],
                                     start=True, stop=True)
                    rb = artmp.tile([128, 512], f32, tag="rb")
                    nc.vector.tensor_copy(rb[:], rbps[:])
                    nc.vector.tensor_tensor(o_sb[(b, h)][:], ov[:], rb[:],
                                            op=ALU.mult)

        # ================= Phase C: o-proj + LN + FFN =================
        with tc.tile_pool(name="ctmp", bufs=3) as ctmp, \
             tc.tile_pool(name="cres", bufs=3) as cres, \
             tc.tile_pool(name="cps", bufs=2, space="PSUM") as cps:

            wo = [wot[:, ct, :] for ct in range(NCT)]
            wf = [wft[:, ct, :] for ct in range(NCT)]

            for b in range(BLOC):
                for t4 in range(T // 128):
                    tt = b * (T // 128) + t4
                    row0 = tt * 128
                    x2t = ctmp.tile([128, DIM], f32, tag="x2")
                    # o-proj into x2 (+residual incl. b_o on DVE)
                    for jc in range(2):
                        sl = slice(jc * 512, (jc + 1) * 512)
                        ps = cps.tile([128, 512], f32, tag="op", bufs=3)
                        for h in range(NH):
                            nc.tensor.matmul(
                                ps[:], o_sb[(b, h)][:, t4 * 128:(t4 + 1) * 128],
                                wo[h][:, sl], start=(h == 0), stop=(h == NH - 1))
                        nc.vector.tensor_tensor(x2t[:, sl], ps[:],
                                                xn_all[:, tt, sl], op=ALU.add)
                    # LayerNorm stats
                    stats = ctmp.tile([128, 2, 6], f32, tag="stats")
                    nc.vector.bn_stats(stats[:, 0, :], x2t[:, 0:512])
                    nc.vector.bn_stats(stats[:, 1, :], x2t[:, 512:1024])
                    mv = ctmp.tile([128, 2], f32, tag="mv")
                    nc.vector.bn_aggr(mv[:], stats[:])
                    rstd = ctmp.tile([128, 1], f32, tag="rstd")
                    nc.scalar.activation(rstd[:], mv[:, 1:2], AF.Sqrt,
                                         bias=sb_eps[:])
                    rstd2 = ctmp.tile([128, 1], f32, tag="rstd2")
                    nc.vector.reciprocal(rstd2[:], rstd[:])
                    z = ctmp.tile([128, DIM], bf16, tag="z")
                    nc.vector.tensor_scalar(z[:], x2t[:],
                                            scalar1=mv[:, 0:1], scalar2=rstd2[:],
                                            op0=ALU.subtract, op1=ALU.mult)
                    # transpose z -> zT (2 halves of 4 blocks each)
                    zT = []
                    for half in range(2):
                        tp = cps.tile([128, 512], bf16, tag="tp")
                        for q in range(4):
                            cb = half * 4 + q
                            nc.tensor.transpose(
                                tp[:, q * 128:(q + 1) * 128],
                                z[:, cb * 128:(cb + 1) * 128], sb_ident[:])
                        zt = ctmp.tile([128, 512], bf16, tag=f"zT{half}")
                        nc.vector.tensor_copy(zt[:], tp[:])
                        zT.append(zt)
                    # FFN + ReLU + store
                    for jc in range(2):
                        sl = slice(jc * 512, (jc + 1) * 512)
                        fp = cps.tile([128, 512], f32, tag="fp")
                        for ct in range(NCT):
                            nc.tensor.matmul(
                                fp[:], zT[ct // 4][:, (ct % 4) * 128:(ct % 4 + 1) * 128],
                                wf[ct][:, sl], start=(ct == 0), stop=False)
                        nc.tensor.matmul(fp[:], sb_ones_row[:], sb_bf[:, sl],
                                         start=False, stop=True)
                        res = cres.tile([128, 512], f32, tag="res")
                        nc.scalar.activation(res[:], fp[:], AF.Relu)
                        nc.sync.dma_start(out_d.ap()[row0:row0 + 128, sl], res[:])

    nc.compile()
    return nc


def _prep_host(inputs):
    """Host-side preprocessing: expert select, folding, transposes, sharding."""
    x = np.asarray(inputs["x"], dtype=np.float32)
    h_a = np.asarray(inputs["h_a"], dtype=np.float32)
    h_t = np.asarray(inputs["h_t"], dtype=np.float32)
    e = int(np.asarray(inputs["expert_idx"]))
    g = float(1.0 / (1.0 + math.exp(-float(np.asarray(inputs["gating_factor"])[e]))))
    sc = 1.0 / math.sqrt(HD)

    def wT(w, scale=1.0):
        return np.ascontiguousarray(
            (np.asarray(w, dtype=np.float32)[e] * scale).T).astype(BF16)

    def brow(bv, scale=1.0, dtype=BF16):
        return (np.asarray(bv, dtype=np.float32)[e] * scale).reshape(1, DIM).astype(dtype)

    def bcol(bv, scale=1.0):
        # [DIM] -> [128, NH]: column h = b[h*128:(h+1)*128]
        return np.ascontiguousarray(
            (np.asarray(bv, dtype=np.float32)[e] * scale).reshape(NH, 128).T
        ).astype(np.float32)

    gamma = np.asarray(inputs["gamma"], dtype=np.float32)[e]
    beta = np.asarray(inputs["beta"], dtype=np.float32)[e]
    w_ffn = np.asarray(inputs["W_ffn"], dtype=np.float32)[e]
    b_ffn = np.asarray(inputs["b_ffn"], dtype=np.float32)[e]
    w_f_eff = w_ffn * gamma[None, :]
    b_f_eff = b_ffn + w_ffn @ beta

    shared = {
        "wqaT": wT(inputs["W_qa"], sc),
        "wqtT": wT(inputs["W_qt"], sc * g),
        "wkaT": wT(inputs["W_ka"]),
        "wktT": wT(inputs["W_kt"]),
        "wvaT": wT(inputs["W_va"]),
        "wvtT": wT(inputs["W_vt"]),
        "woT": wT(inputs["W_o"]),
        "wfT": np.ascontiguousarray(w_f_eff.T).astype(BF16),
        "biascols": np.ascontiguousarray(np.concatenate([
            bcol(inputs["b_qa"], sc),
            bcol(inputs["b_qt"], sc * g),
            bcol(inputs["b_ka"]),
            bcol(inputs["b_kt"]),
        ], axis=1)),
        "bva_b": np.ascontiguousarray(np.tile(
            np.asarray(inputs["b_va"], dtype=np.float32)[e][None, :], (128, 1))),
        "bvt_b": np.ascontiguousarray(np.tile(
            np.asarray(inputs["b_vt"], dtype=np.float32)[e][None, :], (128, 1))),
        "bf_row": b_f_eff.reshape(1, DIM).astype(BF16),
    }
    b_o = np.asarray(inputs["b_o"], dtype=np.float32)[e]

    in_maps = []
    for c in range(NCORES):
        xc = x[c * BLOC:(c + 1) * BLOC].reshape(TOKQ, DIM)
        hac = h_a[c * BLOC:(c + 1) * BLOC].reshape(TOKK, DIM)
        htc = h_t[c * BLOC:(c + 1) * BLOC].reshape(TOKK, DIM)
        m = dict(shared)
        m["xT"] = np.ascontiguousarray(xc.T).astype(BF16)
        m["xnat"] = np.ascontiguousarray(xc + b_o[None, :])
        m["haT"] = np.ascontiguousarray(hac.T).astype(BF16)
        m["htT"] = np.ascontiguousarray(htc.T).astype(BF16)
        in_maps.append(m)
    return in_maps


def run(inputs, trace=False):
    from concourse.bass_utils import run_bass_kernel_spmd

    if "nc" not in _CACHE:
        _CACHE["nc"] = build_program()
    nc = _CACHE["nc"]
    in_maps = _prep_host(inputs)
    res = run_bass_kernel_spmd(nc, in_maps, list(range(NCORES)), trace=trace)
    outs = [res.results[c]["out"].reshape(BLOC, T, DIM) for c in range(NCORES)]
    return np.concatenate(outs, axis=0), res


def kernel(**inputs) -> np.ndarray:
    out, _ = run(inputs, trace=False)
    return out

